# revision 12
# baseline (speedup 1.0000x reference)
"""Causal single-head attention (B=4, S=2048, D=1024, fp32) on 8 TRN2 NeuronCores.

Sharding: core c <-> (batch c//2, parity c%2). Each core owns the 8 even or odd
128-row query tiles of its batch (balanced causal work), computes Q/K/V
projections locally (K/V duplicated within the batch pair), and runs
flash-style causal attention over its query tiles. Key dimension is processed
in two 1024-key passes to fit SBUF. All matmuls run in fp32r (full PE rate);
inputs are pre-rounded to the fp32r grid and laid out (x transposed, weights
packed per e-tile) on the host so every DMA is a plain contiguous HWDGE copy.

Self-contained: hardcodes shapes; reads nothing from disk.
"""
import sys

import numpy as np

try:
    from concourse import bass, bacc, tile
except ImportError:  # concourse ships with the container, not this file
    for _p in ("/opt/trn_rl_repo", "/root/.axon_site/_ro/trn_rl_repo"):
        if _p not in sys.path:
            sys.path.append(_p)
    from concourse import bass, bacc, tile
from concourse import mybir
from concourse.bass_utils import run_bass_kernel_spmd

dt = mybir.dt
AF = mybir.ActivationFunctionType

B, S, D = 4, 2048, 1024
P = 128
ND = D // P          # 8 d-tiles (contraction of projections)
NE = D // P          # 8 e-tiles (output feature tiles)
SLOTS = 8            # q-tiles per core
HT = 8               # k-tiles per pass
NCORES = 8
SCALE = 1.0 / float(np.sqrt(D))
NEG = -1.0e30


def _emit_body(nc, tc, pools, aps):
    (sb_const, sb_xt, sb_qt, sb_kt, sb_vt, sb_wblk, sb_wv,
     sb_p, sb_pt, sb_sums, sb_opart, sb_obuf, sb_ojoin, ps) = pools
    xT, xqT, Wq, Wk, Wv, mask, ident, O = aps

    ident_r = sb_const.tile([P, P], dt.float32r, tag="ident", name="ident_r")
    nc.sync.dma_start(ident_r[:], ident[:])
    mask_sb = sb_const.tile([P, 2 * P], dt.float32, tag="mask", name="mask_sb")
    nc.sync.dma_start(mask_sb[:], mask[:])

    # ---- xq^T: own-query columns of x^T (host-transposed)
    xqt = [sb_xt.tile([P, SLOTS * P], dt.float32r, tag=f"xt{d}", name=f"xqt{d}")
           for d in range(ND)]
    for d in range(ND):
        nc.sync.dma_start(xqt[d][:], xqT[d * P:(d + 1) * P, :])

    # ---- Q^T projection: qt[e][:, q] = sum_d Wq[d,e]^T x^T[d,q]
    qt = [sb_qt.tile([P, SLOTS * P], dt.float32r, tag=f"qt{e}", name=f"qt{e}")
          for e in range(NE)]
    for e in range(NE):
        wb = sb_wblk.tile([P, D], dt.float32r, tag="wblk", name="wb")
        nc.sync.dma_start(wb[:], Wq[e * P:(e + 1) * P, :])
        qp0 = ps.tile([P, 512], dt.float32, tag="proj", name="qp0")
        qp1 = ps.tile([P, 512], dt.float32, tag="proj", name="qp1")
        for d in range(ND):
            nc.tensor.matmul(qp0[:], wb[:, d * P:(d + 1) * P], xqt[d][:, 0:512],
                             start=(d == 0), stop=(d == ND - 1))
            nc.tensor.matmul(qp1[:], wb[:, d * P:(d + 1) * P], xqt[d][:, 512:1024],
                             start=(d == 0), stop=(d == ND - 1))
        nc.vector.tensor_copy(qt[e][:, 0:512], qp0[:])
        nc.vector.tensor_copy(qt[e][:, 512:1024], qp1[:])

    # per-slot softmax chunk sums + norm scratch (cols 4j..4j+3 sums; 32+2j ssum; 33+2j rec)
    sums = sb_sums.tile([P, 48], dt.float32, tag="sums", name="sums")
    opart_d = sb_opart.tile([4 * P, D], dt.float32, tag="opartd", name="opartd")
    chunk_ctr = [0] * SLOTS

    for p in range(2):
        # ---- x^T for this pass's keys (direct DMA from host-transposed x)
        xt = [sb_xt.tile([P, HT * P], dt.float32r, tag=f"xt{d}", name=f"xt{d}")
              for d in range(ND)]
        for d in range(ND):
            nc.sync.dma_start(xt[d][:], xT[d * P:(d + 1) * P, p * 1024:(p + 1) * 1024])

        # ---- K^T projection for this pass
        kt = [sb_kt.tile([P, HT * P], dt.float32r, tag=f"kt{e}", name=f"kt{e}")
              for e in range(NE)]
        for e in range(NE):
            wb = sb_wblk.tile([P, D], dt.float32r, tag="wblk", name="wb")
            nc.sync.dma_start(wb[:], Wk[e * P:(e + 1) * P, :])
            kp0 = ps.tile([P, 512], dt.float32, tag="proj", name="kp0")
            kp1 = ps.tile([P, 512], dt.float32, tag="proj", name="kp1")
            for d in range(ND):
                nc.tensor.matmul(kp0[:], wb[:, d * P:(d + 1) * P], xt[d][:, 0:512],
                                 start=(d == 0), stop=(d == ND - 1))
                nc.tensor.matmul(kp1[:], wb[:, d * P:(d + 1) * P], xt[d][:, 512:1024],
                                 start=(d == 0), stop=(d == ND - 1))
            nc.vector.tensor_copy(kt[e][:, 0:512], kp0[:])
            nc.vector.tensor_copy(kt[e][:, 512:1024], kp1[:])

        # ---- V projection for this pass (natural [s, e] layout)
        vt = [sb_vt.tile([P, D], dt.float32r, tag=f"vt{st}", name=f"vt{st}")
              for st in range(HT)]
        for ec in range(2):
            wv = sb_wv.tile([P, ND * 512], dt.float32r, tag="wv", name="wv")
            nc.sync.dma_start(wv[:], Wv[ec * P:(ec + 1) * P, :])
            for st in range(HT):
                vp = ps.tile([P, 512], dt.float32, tag="proj", name="vp")
                for d in range(ND):
                    nc.tensor.matmul(vp[:], xt[d][:, st * P:(st + 1) * P],
                                     wv[:, d * 512:(d + 1) * 512],
                                     start=(d == 0), stop=(d == ND - 1))
                nc.vector.tensor_copy(vt[st][:, ec * 512:(ec + 1) * 512], vp[:])

        # ---- attention for each slot against this pass's keys
        for j in range(SLOTS):
            ext = 2 * j + 2                      # causal extent in k-tiles (uniform)
            t0 = HT * p
            t1 = min(ext, HT * (p + 1))
            if t1 <= t0:
                continue
            final_pass = (ext - 1) // HT == p

            o_ps = [ps.tile([P, 512], dt.float32, tag="o", name=f"ops{ec}")
                    for ec in range(2)]

            c = t0 * P
            while c < t1 * P:
                cw = 512 if t1 * P - c >= 512 else t1 * P - c
                sps = ps.tile([P, cw], dt.float32, tag="s", name="sps")
                for e in range(NE):
                    nc.tensor.matmul(sps[:], qt[e][:, j * P:(j + 1) * P],
                                     kt[e][:, c - p * 1024: c - p * 1024 + cw],
                                     start=(e == 0), stop=(e == NE - 1))
                if final_pass and c + cw == ext * P:
                    off = cw - 2 * P
                    nc.vector.tensor_add(sps[:, off:off + 2 * P], sps[:, off:off + 2 * P], mask_sb[:])
                pch = sb_p.tile([P, cw], dt.float32r, tag="p", name="pch")
                ci = chunk_ctr[j]
                chunk_ctr[j] += 1
                nc.scalar.activation(pch[:], sps[:], AF.Exp, scale=SCALE,
                                     accum_out=sums[:, 4 * j + ci:4 * j + ci + 1])
                for ti in range(cw // P):
                    t = c // P + ti
                    ptp = ps.tile([P, P], dt.float32, tag="tr", name="ptp")
                    nc.tensor.transpose(ptp[:].bitcast(dt.float32r), pch[:, ti * P:(ti + 1) * P], ident_r[:])
                    ptb = sb_pt.tile([P, P], dt.float32r, tag="pt", name="ptb")
                    nc.vector.tensor_copy(ptb[:], ptp[:].bitcast(dt.float32r))
                    for ec in range(2):
                        nc.tensor.matmul(o_ps[ec][:], ptb[:],
                                         vt[t - HT * p][:, ec * 512:(ec + 1) * 512],
                                         start=(t == t0), stop=(t == t1 - 1))
                c += cw

            if final_pass:
                ssum = sums[:, 32 + 2 * j:33 + 2 * j]
                nc.vector.reduce_sum(ssum, sums[:, 4 * j:4 * j + chunk_ctr[j]],
                                     axis=mybir.AxisListType.X)
                rec = sums[:, 33 + 2 * j:34 + 2 * j]
                nc.vector.reciprocal(rec, ssum)
                ob = sb_obuf.tile([P, D], dt.float32, tag="obuf", name="ob")
                for ec in range(2):
                    if ext > HT:
                        oj = sb_ojoin.tile([P, 512], dt.float32, tag="ojoin", name="oj")
                        nc.sync.dma_start(oj[:], opart_d[(j - 4) * P:(j - 3) * P, ec * 512:(ec + 1) * 512])
                        nc.vector.tensor_add(oj[:], o_ps[ec][:], oj[:])
                        nc.scalar.mul(ob[:, ec * 512:(ec + 1) * 512], oj[:], rec)
                    else:
                        nc.scalar.mul(ob[:, ec * 512:(ec + 1) * 512], o_ps[ec][:], rec)
                nc.sync.dma_start(O[j * P:(j + 1) * P, :], ob[:])
            else:
                ob = sb_obuf.tile([P, D], dt.float32, tag="obuf", name="ob")
                for ec in range(2):
                    nc.vector.tensor_copy(ob[:, ec * 512:(ec + 1) * 512], o_ps[ec][:])
                nc.sync.dma_start(opart_d[(j - 4) * P:(j - 3) * P, :], ob[:])


def build_program(reps: int = 1):
    nc = bacc.Bacc("TRN2", target_bir_lowering=False, debug=False, num_devices=NCORES)

    xT_t = nc.dram_tensor("xT", [D, S], dt.float32r, kind="ExternalInput")
    xqT_t = nc.dram_tensor("xqT", [D, SLOTS * P], dt.float32r, kind="ExternalInput")
    Wq_t = nc.dram_tensor("Wq", [D, D], dt.float32r, kind="ExternalInput")
    Wk_t = nc.dram_tensor("Wk", [D, D], dt.float32r, kind="ExternalInput")
    Wv_t = nc.dram_tensor("Wv", [2 * P, ND * 512], dt.float32r, kind="ExternalInput")
    mask_t = nc.dram_tensor("mask", [P, 2 * P], dt.float32, kind="ExternalInput")
    ident_t = nc.dram_tensor("ident", [P, P], dt.float32r, kind="ExternalInput")
    O_t = nc.dram_tensor("O", [SLOTS * P, D], dt.float32, kind="ExternalOutput")

    aps = (xT_t.ap(), xqT_t.ap(), Wq_t.ap(), Wk_t.ap(), Wv_t.ap(), mask_t.ap(),
           ident_t.ap(), O_t.ap())

    with tile.TileContext(nc) as tc:
        with (
            tc.tile_pool(name="const", bufs=1) as sb_const,
            tc.tile_pool(name="xt", bufs=1) as sb_xt,
            tc.tile_pool(name="qt", bufs=1) as sb_qt,
            tc.tile_pool(name="kt", bufs=1) as sb_kt,
            tc.tile_pool(name="vt", bufs=1) as sb_vt,
            tc.tile_pool(name="wblk", bufs=3) as sb_wblk,
            tc.tile_pool(name="wv", bufs=1) as sb_wv,
            tc.tile_pool(name="p", bufs=3) as sb_p,
            tc.tile_pool(name="pt", bufs=6) as sb_pt,
            tc.tile_pool(name="sums", bufs=1) as sb_sums,
            tc.tile_pool(name="opart", bufs=1, space=bass.MemorySpace.DRAM) as sb_opart,
            tc.tile_pool(name="obuf", bufs=2) as sb_obuf,
            tc.tile_pool(name="ojoin", bufs=2) as sb_ojoin,
            tc.tile_pool(name="ps", bufs=2, space=bass.MemorySpace.PSUM) as ps,
        ):
            pools = (sb_const, sb_xt, sb_qt, sb_kt, sb_vt,
                     sb_wblk, sb_wv, sb_p, sb_pt, sb_sums, sb_opart,
                     sb_obuf, sb_ojoin, ps)
            if reps == 1:
                _emit_body(nc, tc, pools, aps)
            else:
                with tc.For_i(0, reps, 1):
                    _emit_body(nc, tc, pools, aps)

    nc.compile()
    return nc


def round_f32r(a):
    """Round fp32 to the fp32r grid (low 12 mantissa bits dropped, nearest-even)."""
    u = np.ascontiguousarray(a, np.float32).view(np.uint32).copy()
    low = u & np.uint32(0xFFF)
    base = u & np.uint32(0xFFFFF000)
    up = (low > 0x800) | ((low == 0x800) & (((base >> 12) & 1) == 1))
    base[up] += np.uint32(0x1000)
    return base.view(np.float32)


def pack_w_cols(W, cw):
    """[D, D] -> [(D//cw)*P, ND*cw]: row (e*P+p), col (d*cw+c) = W[d*P+p, e*cw+c]."""
    ne = D // cw
    return np.ascontiguousarray(
        W.reshape(ND, P, ne, cw).transpose(2, 1, 0, 3).reshape(ne * P, ND * cw))


def make_in_maps(x, Wq, Wk, Wv):
    x = round_f32r(x.reshape(B, S, D))
    Wq = pack_w_cols(round_f32r(Wq), P)
    Wk = pack_w_cols(round_f32r(Wk), P)
    Wv = pack_w_cols(round_f32r(Wv), 512)
    ident = np.eye(P, dtype=np.float32)
    tri = np.where(np.arange(P)[None, :] <= np.arange(P)[:, None], 0.0, NEG).astype(np.float32)
    masks = [
        np.concatenate([tri, np.full((P, P), NEG, np.float32)], axis=1),   # parity 0
        np.concatenate([np.zeros((P, P), np.float32), tri], axis=1),       # parity 1
    ]
    xT = [np.ascontiguousarray(x[b].T) for b in range(B)]   # [D, S] per batch
    in_maps = []
    for c in range(NCORES):
        b, r = c // 2, c % 2
        xTb = xT[b]
        cols = np.concatenate([np.arange((2 * j + r) * P, (2 * j + r + 1) * P)
                               for j in range(SLOTS)])
        xqTb = np.ascontiguousarray(xTb[:, cols])
        in_maps.append({
            "xT": xTb, "xqT": xqTb, "Wq": Wq, "Wk": Wk, "Wv": Wv,
            "mask": masks[r], "ident": ident,
        })
    return in_maps


def assemble_output(results):
    out = np.empty((B, S, D), dtype=np.float32)
    for c in range(NCORES):
        b, r = c // 2, c % 2
        oc = results[c]["O"].reshape(SLOTS, P, D)
        for j in range(SLOTS):
            out[b, (2 * j + r) * P:(2 * j + r + 1) * P, :] = oc[j]
    return out


_nc_cache = {}


def _get_program(reps: int = 1):
    if reps not in _nc_cache:
        _nc_cache[reps] = build_program(reps)
    return _nc_cache[reps]


def kernel(x, Wq, Wk, Wv):
    x = np.asarray(x, dtype=np.float32)
    Wq = np.asarray(Wq, dtype=np.float32)
    Wk = np.asarray(Wk, dtype=np.float32)
    Wv = np.asarray(Wv, dtype=np.float32)
    nc = _get_program(1)
    in_maps = make_in_maps(x, Wq, Wk, Wv)
    results = run_bass_kernel_spmd(nc, in_maps, list(range(NCORES))).results
    return assemble_output(results)


# revision 15
# speedup vs baseline: 1.0095x; 1.0095x over previous
"""Causal single-head attention (B=4, S=2048, D=1024, fp32) on 8 TRN2 NeuronCores.

Sharding: core c <-> (batch c//2, parity c%2). Each core owns the 8 even or odd
128-row query tiles of its batch (balanced causal work), computes Q/K/V
projections locally (K/V duplicated within the batch pair), and runs
flash-style causal attention over its query tiles. Key dimension is processed
in two 1024-key passes to fit SBUF. All matmuls run in fp32r (full PE rate);
inputs are pre-rounded to the fp32r grid and laid out (x transposed, weights
packed per e-tile) on the host so every DMA is a plain contiguous HWDGE copy.

Self-contained: hardcodes shapes; reads nothing from disk.
"""
import sys

import numpy as np

try:
    from concourse import bass, bacc, tile
except ImportError:  # concourse ships with the container, not this file
    for _p in ("/opt/trn_rl_repo", "/root/.axon_site/_ro/trn_rl_repo"):
        if _p not in sys.path:
            sys.path.append(_p)
    from concourse import bass, bacc, tile
from concourse import mybir
from concourse.bass_utils import run_bass_kernel_spmd

dt = mybir.dt
AF = mybir.ActivationFunctionType

B, S, D = 4, 2048, 1024
P = 128
ND = D // P          # 8 d-tiles (contraction of projections)
NE = D // P          # 8 e-tiles (output feature tiles)
SLOTS = 8            # q-tiles per core
HT = 8               # k-tiles per pass
NCORES = 8
SCALE = 1.0 / float(np.sqrt(D))
NEG = -1.0e30


def _emit_body(nc, tc, pools, aps):
    (sb_const, sb_xt, sb_qt, sb_kt, sb_vt, sb_wblk, sb_wv,
     sb_p, sb_pt, sb_sums, sb_opart, sb_obuf, sb_ojoin, ps) = pools
    xT, xqT, Wq, Wk, Wv, mask, ident, O = aps

    ident_r = sb_const.tile([P, P], dt.float32r, tag="ident", name="ident_r")
    nc.sync.dma_start(ident_r[:], ident[:])
    mask_sb = sb_const.tile([P, 2 * P], dt.float32, tag="mask", name="mask_sb")
    nc.sync.dma_start(mask_sb[:], mask[:])

    # ---- xq^T: own-query columns of x^T (host-transposed)
    xqt = [sb_xt.tile([P, SLOTS * P], dt.float32r, tag=f"xt{d}", name=f"xqt{d}")
           for d in range(ND)]
    wb0 = sb_wblk.tile([P, D], dt.float32r, tag="wblk", name="wb0")
    nc.sync.dma_start(xqt[0][:, 0:512], xqT[0:P, 0:512])
    nc.sync.dma_start(wb0[:], Wq[0:P, :])
    nc.sync.dma_start(xqt[0][:, 512:1024], xqT[0:P, 512:1024])
    for d in range(1, ND):
        for h in range(2):
            nc.sync.dma_start(xqt[d][:, h * 512:(h + 1) * 512],
                              xqT[d * P:(d + 1) * P, h * 512:(h + 1) * 512])

    # ---- Q^T projection: qt[e][:, q] = sum_d Wq[d,e]^T x^T[d,q]
    qt = [sb_qt.tile([P, SLOTS * P], dt.float32r, tag=f"qt{e}", name=f"qt{e}")
          for e in range(NE)]
    for e in range(NE):
        if e == 0:
            wb = wb0
        else:
            wb = sb_wblk.tile([P, D], dt.float32r, tag="wblk", name="wb")
            nc.sync.dma_start(wb[:], Wq[e * P:(e + 1) * P, :])
        qp0 = ps.tile([P, 512], dt.float32, tag="proj", name="qp0")
        qp1 = ps.tile([P, 512], dt.float32, tag="proj", name="qp1")
        for d in range(ND):
            nc.tensor.matmul(qp0[:], wb[:, d * P:(d + 1) * P], xqt[d][:, 0:512],
                             start=(d == 0), stop=(d == ND - 1))
            nc.tensor.matmul(qp1[:], wb[:, d * P:(d + 1) * P], xqt[d][:, 512:1024],
                             start=(d == 0), stop=(d == ND - 1))
        nc.vector.tensor_copy(qt[e][:, 0:512], qp0[:])
        nc.vector.tensor_copy(qt[e][:, 512:1024], qp1[:])

    # per-slot softmax chunk sums + norm scratch (cols 4j..4j+3 sums; 32+2j ssum; 33+2j rec)
    sums = sb_sums.tile([P, 48], dt.float32, tag="sums", name="sums")
    opart_d = sb_opart.tile([4 * P, D], dt.float32, tag="opartd", name="opartd")
    chunk_ctr = [0] * SLOTS

    for p in range(2):
        # ---- x^T for this pass's keys (direct DMA from host-transposed x)
        xt = [sb_xt.tile([P, HT * P], dt.float32r, tag=f"xt{d}", name=f"xt{d}")
              for d in range(ND)]
        for d in range(ND):
            for h in range(2):
                nc.sync.dma_start(xt[d][:, h * 512:(h + 1) * 512],
                                  xT[d * P:(d + 1) * P, p * 1024 + h * 512:p * 1024 + (h + 1) * 512])

        # ---- K^T projection for this pass
        kt = [sb_kt.tile([P, HT * P], dt.float32r, tag=f"kt{e}", name=f"kt{e}")
              for e in range(NE)]
        for e in range(NE):
            wb = sb_wblk.tile([P, D], dt.float32r, tag="wblk", name="wb")
            nc.sync.dma_start(wb[:], Wk[e * P:(e + 1) * P, :])
            kp0 = ps.tile([P, 512], dt.float32, tag="proj", name="kp0")
            kp1 = ps.tile([P, 512], dt.float32, tag="proj", name="kp1")
            for d in range(ND):
                nc.tensor.matmul(kp0[:], wb[:, d * P:(d + 1) * P], xt[d][:, 0:512],
                                 start=(d == 0), stop=(d == ND - 1))
                nc.tensor.matmul(kp1[:], wb[:, d * P:(d + 1) * P], xt[d][:, 512:1024],
                                 start=(d == 0), stop=(d == ND - 1))
            nc.vector.tensor_copy(kt[e][:, 0:512], kp0[:])
            nc.vector.tensor_copy(kt[e][:, 512:1024], kp1[:])

        # ---- V projection for this pass (natural [s, e] layout)
        vt = [sb_vt.tile([P, D], dt.float32r, tag=f"vt{st}", name=f"vt{st}")
              for st in range(HT)]
        for ec in range(2):
            wv = sb_wv.tile([P, ND * 512], dt.float32r, tag="wv", name="wv")
            for d in range(ND):
                nc.sync.dma_start(wv[:, d * 512:(d + 1) * 512],
                                  Wv[ec * P:(ec + 1) * P, d * 512:(d + 1) * 512])
            for st in range(HT):
                vp = ps.tile([P, 512], dt.float32, tag="proj", name="vp")
                for d in range(ND):
                    nc.tensor.matmul(vp[:], xt[d][:, st * P:(st + 1) * P],
                                     wv[:, d * 512:(d + 1) * 512],
                                     start=(d == 0), stop=(d == ND - 1))
                nc.vector.tensor_copy(vt[st][:, ec * 512:(ec + 1) * 512], vp[:])

        # ---- attention for each slot against this pass's keys
        slot_order = list(range(SLOTS)) if p == 0 else [7, 6, 5, 4]
        for j in slot_order:
            ext = 2 * j + 2                      # causal extent in k-tiles (uniform)
            t0 = HT * p
            t1 = min(ext, HT * (p + 1))
            if t1 <= t0:
                continue
            final_pass = (ext - 1) // HT == p

            o_ps = [ps.tile([P, 512], dt.float32, tag="o", name=f"ops{ec}")
                    for ec in range(2)]

            c = t0 * P
            while c < t1 * P:
                cw = 512 if t1 * P - c >= 512 else t1 * P - c
                sps = ps.tile([P, cw], dt.float32, tag="s", name="sps")
                for e in range(NE):
                    nc.tensor.matmul(sps[:], qt[e][:, j * P:(j + 1) * P],
                                     kt[e][:, c - p * 1024: c - p * 1024 + cw],
                                     start=(e == 0), stop=(e == NE - 1))
                if final_pass and c + cw == ext * P:
                    off = cw - 2 * P
                    nc.vector.tensor_add(sps[:, off:off + 2 * P], sps[:, off:off + 2 * P], mask_sb[:])
                pch = sb_p.tile([P, cw], dt.float32r, tag="p", name="pch")
                ci = chunk_ctr[j]
                chunk_ctr[j] += 1
                nc.scalar.activation(pch[:], sps[:], AF.Exp, scale=SCALE,
                                     accum_out=sums[:, 4 * j + ci:4 * j + ci + 1])
                for ti in range(cw // P):
                    t = c // P + ti
                    ptp = ps.tile([P, P], dt.float32, tag="tr", name="ptp")
                    nc.tensor.transpose(ptp[:].bitcast(dt.float32r), pch[:, ti * P:(ti + 1) * P], ident_r[:])
                    ptb = sb_pt.tile([P, P], dt.float32r, tag="pt", name="ptb")
                    nc.vector.tensor_copy(ptb[:], ptp[:].bitcast(dt.float32r))
                    for ec in range(2):
                        nc.tensor.matmul(o_ps[ec][:], ptb[:],
                                         vt[t - HT * p][:, ec * 512:(ec + 1) * 512],
                                         start=(t == t0), stop=(t == t1 - 1))
                c += cw

            if final_pass:
                ssum = sums[:, 32 + 2 * j:33 + 2 * j]
                nc.vector.reduce_sum(ssum, sums[:, 4 * j:4 * j + chunk_ctr[j]],
                                     axis=mybir.AxisListType.X)
                rec = sums[:, 33 + 2 * j:34 + 2 * j]
                nc.vector.reciprocal(rec, ssum)
                ob = sb_obuf.tile([P, D], dt.float32, tag="obuf", name="ob")
                for ec in range(2):
                    if ext > HT:
                        oj = sb_ojoin.tile([P, 512], dt.float32, tag="ojoin", name="oj")
                        nc.sync.dma_start(oj[:], opart_d[(j - 4) * P:(j - 3) * P, ec * 512:(ec + 1) * 512])
                        nc.vector.tensor_add(oj[:], o_ps[ec][:], oj[:])
                        nc.scalar.mul(ob[:, ec * 512:(ec + 1) * 512], oj[:], rec)
                    else:
                        nc.scalar.mul(ob[:, ec * 512:(ec + 1) * 512], o_ps[ec][:], rec)
                nc.sync.dma_start(O[j * P:(j + 1) * P, :], ob[:])
            else:
                ob = sb_obuf.tile([P, D], dt.float32, tag="obuf", name="ob")
                for ec in range(2):
                    nc.vector.tensor_copy(ob[:, ec * 512:(ec + 1) * 512], o_ps[ec][:])
                nc.sync.dma_start(opart_d[(j - 4) * P:(j - 3) * P, :], ob[:])


def build_program(reps: int = 1):
    nc = bacc.Bacc("TRN2", target_bir_lowering=False, debug=False, num_devices=NCORES)

    xT_t = nc.dram_tensor("xT", [D, S], dt.float32r, kind="ExternalInput")
    xqT_t = nc.dram_tensor("xqT", [D, SLOTS * P], dt.float32r, kind="ExternalInput")
    Wq_t = nc.dram_tensor("Wq", [D, D], dt.float32r, kind="ExternalInput")
    Wk_t = nc.dram_tensor("Wk", [D, D], dt.float32r, kind="ExternalInput")
    Wv_t = nc.dram_tensor("Wv", [2 * P, ND * 512], dt.float32r, kind="ExternalInput")
    mask_t = nc.dram_tensor("mask", [P, 2 * P], dt.float32, kind="ExternalInput")
    ident_t = nc.dram_tensor("ident", [P, P], dt.float32r, kind="ExternalInput")
    O_t = nc.dram_tensor("O", [SLOTS * P, D], dt.float32, kind="ExternalOutput")

    aps = (xT_t.ap(), xqT_t.ap(), Wq_t.ap(), Wk_t.ap(), Wv_t.ap(), mask_t.ap(),
           ident_t.ap(), O_t.ap())

    with tile.TileContext(nc) as tc:
        with (
            tc.tile_pool(name="const", bufs=1) as sb_const,
            tc.tile_pool(name="xt", bufs=1) as sb_xt,
            tc.tile_pool(name="qt", bufs=1) as sb_qt,
            tc.tile_pool(name="kt", bufs=1) as sb_kt,
            tc.tile_pool(name="vt", bufs=1) as sb_vt,
            tc.tile_pool(name="wblk", bufs=3) as sb_wblk,
            tc.tile_pool(name="wv", bufs=1) as sb_wv,
            tc.tile_pool(name="p", bufs=3) as sb_p,
            tc.tile_pool(name="pt", bufs=6) as sb_pt,
            tc.tile_pool(name="sums", bufs=1) as sb_sums,
            tc.tile_pool(name="opart", bufs=1, space=bass.MemorySpace.DRAM) as sb_opart,
            tc.tile_pool(name="obuf", bufs=3) as sb_obuf,
            tc.tile_pool(name="ojoin", bufs=4) as sb_ojoin,
            tc.tile_pool(name="ps", bufs=2, space=bass.MemorySpace.PSUM) as ps,
        ):
            pools = (sb_const, sb_xt, sb_qt, sb_kt, sb_vt,
                     sb_wblk, sb_wv, sb_p, sb_pt, sb_sums, sb_opart,
                     sb_obuf, sb_ojoin, ps)
            if reps == 1:
                _emit_body(nc, tc, pools, aps)
            else:
                with tc.For_i(0, reps, 1):
                    _emit_body(nc, tc, pools, aps)

    nc.compile()
    return nc


def round_f32r(a):
    """Round fp32 to the fp32r grid (low 12 mantissa bits dropped, nearest-even)."""
    u = np.ascontiguousarray(a, np.float32).view(np.uint32).copy()
    low = u & np.uint32(0xFFF)
    base = u & np.uint32(0xFFFFF000)
    up = (low > 0x800) | ((low == 0x800) & (((base >> 12) & 1) == 1))
    base[up] += np.uint32(0x1000)
    return base.view(np.float32)


def pack_w_cols(W, cw):
    """[D, D] -> [(D//cw)*P, ND*cw]: row (e*P+p), col (d*cw+c) = W[d*P+p, e*cw+c]."""
    ne = D // cw
    return np.ascontiguousarray(
        W.reshape(ND, P, ne, cw).transpose(2, 1, 0, 3).reshape(ne * P, ND * cw))


def make_in_maps(x, Wq, Wk, Wv):
    x = round_f32r(x.reshape(B, S, D))
    Wq = pack_w_cols(round_f32r(Wq), P)
    Wk = pack_w_cols(round_f32r(Wk), P)
    Wv = pack_w_cols(round_f32r(Wv), 512)
    ident = np.eye(P, dtype=np.float32)
    tri = np.where(np.arange(P)[None, :] <= np.arange(P)[:, None], 0.0, NEG).astype(np.float32)
    masks = [
        np.concatenate([tri, np.full((P, P), NEG, np.float32)], axis=1),   # parity 0
        np.concatenate([np.zeros((P, P), np.float32), tri], axis=1),       # parity 1
    ]
    xT = [np.ascontiguousarray(x[b].T) for b in range(B)]   # [D, S] per batch
    in_maps = []
    for c in range(NCORES):
        b, r = c // 2, c % 2
        xTb = xT[b]
        cols = np.concatenate([np.arange((2 * j + r) * P, (2 * j + r + 1) * P)
                               for j in range(SLOTS)])
        xqTb = np.ascontiguousarray(xTb[:, cols])
        in_maps.append({
            "xT": xTb, "xqT": xqTb, "Wq": Wq, "Wk": Wk, "Wv": Wv,
            "mask": masks[r], "ident": ident,
        })
    return in_maps


def assemble_output(results):
    out = np.empty((B, S, D), dtype=np.float32)
    for c in range(NCORES):
        b, r = c // 2, c % 2
        oc = results[c]["O"].reshape(SLOTS, P, D)
        for j in range(SLOTS):
            out[b, (2 * j + r) * P:(2 * j + r + 1) * P, :] = oc[j]
    return out


_nc_cache = {}


def _get_program(reps: int = 1):
    if reps not in _nc_cache:
        _nc_cache[reps] = build_program(reps)
    return _nc_cache[reps]


def kernel(x, Wq, Wk, Wv):
    x = np.asarray(x, dtype=np.float32)
    Wq = np.asarray(Wq, dtype=np.float32)
    Wk = np.asarray(Wk, dtype=np.float32)
    Wv = np.asarray(Wv, dtype=np.float32)
    nc = _get_program(1)
    in_maps = make_in_maps(x, Wq, Wk, Wv)
    results = run_bass_kernel_spmd(nc, in_maps, list(range(NCORES))).results
    return assemble_output(results)


# revision 17
# speedup vs baseline: 1.0887x; 1.0785x over previous
"""Causal single-head attention (B=4, S=2048, D=1024, fp32) on 8 TRN2 NeuronCores.

Sharding: core c <-> (batch c//2, parity c%2). Each core owns the 8 even or odd
128-row query tiles of its batch (balanced causal work), computes Q/K/V
projections locally (K/V duplicated within the batch pair), and runs
flash-style causal attention over its query tiles. Key dimension is processed
in two 1024-key passes to fit SBUF. All matmuls run in fp32r (full PE rate);
inputs are pre-rounded to the fp32r grid and laid out (x transposed, weights
packed per e-tile) on the host so every DMA is a plain contiguous HWDGE copy.

Self-contained: hardcodes shapes; reads nothing from disk.
"""
import sys

import numpy as np

try:
    from concourse import bass, bacc, tile
except ImportError:  # concourse ships with the container, not this file
    for _p in ("/opt/trn_rl_repo", "/root/.axon_site/_ro/trn_rl_repo"):
        if _p not in sys.path:
            sys.path.append(_p)
    from concourse import bass, bacc, tile
from concourse import mybir
from concourse.bass_utils import run_bass_kernel_spmd

dt = mybir.dt
AF = mybir.ActivationFunctionType

B, S, D = 4, 2048, 1024
P = 128
ND = D // P          # 8 d-tiles (contraction of projections)
NE = D // P          # 8 e-tiles (output feature tiles)
SLOTS = 8            # q-tiles per core
HT = 8               # k-tiles per pass
NCORES = 8
SCALE = 1.0 / float(np.sqrt(D))
NEG = -1.0e30


def _emit_body(nc, tc, pools, aps):
    (sb_const, sb_xt, sb_qt, sb_kt, sb_vt, sb_wblk, sb_wv,
     sb_p, sb_pt, sb_sums, sb_opart, sb_obuf, sb_ojoin, ps) = pools
    xT, xqT, Wq, Wk, Wv, mask, ident, O = aps

    ident_r = sb_const.tile([P, P], dt.float32r, tag="ident", name="ident_r")
    nc.sync.dma_start(ident_r[:], ident[:])
    mask_sb = sb_const.tile([P, 2 * P], dt.float32, tag="mask", name="mask_sb")
    nc.sync.dma_start(mask_sb[:], mask[:])

    # ---- xq^T: own-query columns of x^T (host-transposed)
    xqt = [sb_xt.tile([P, SLOTS * P], dt.float32r, tag=f"xt{d}", name=f"xqt{d}")
           for d in range(ND)]
    wb0 = sb_wblk.tile([P, D], dt.float32r, tag="wblk", name="wb0")
    nc.sync.dma_start(xqt[0][:, 0:512], xqT[0:P, 0:512])
    nc.sync.dma_start(wb0[:], Wq[0:P, :])
    nc.sync.dma_start(xqt[0][:, 512:1024], xqT[0:P, 512:1024])
    for d in range(1, ND):
        for h in range(2):
            nc.sync.dma_start(xqt[d][:, h * 512:(h + 1) * 512],
                              xqT[d * P:(d + 1) * P, h * 512:(h + 1) * 512])

    # ---- Q^T projection: qt[e][:, q] = sum_d Wq[d,e]^T x^T[d,q]
    qt = [sb_qt.tile([P, SLOTS * P], dt.float32r, tag=f"qt{e}", name=f"qt{e}")
          for e in range(NE)]
    for e in range(NE):
        if e == 0:
            wb = wb0
        else:
            wb = sb_wblk.tile([P, D], dt.float32r, tag="wblk", name="wb")
            nc.sync.dma_start(wb[:], Wq[e * P:(e + 1) * P, :])
        _tg = "proj" if e % 2 == 0 else "s"
        qp0 = ps.tile([P, 512], dt.float32, tag=_tg, name="qp0")
        qp1 = ps.tile([P, 512], dt.float32, tag=_tg, name="qp1")
        for d in range(ND):
            nc.tensor.matmul(qp0[:], wb[:, d * P:(d + 1) * P], xqt[d][:, 0:512],
                             start=(d == 0), stop=(d == ND - 1))
            nc.tensor.matmul(qp1[:], wb[:, d * P:(d + 1) * P], xqt[d][:, 512:1024],
                             start=(d == 0), stop=(d == ND - 1))
        nc.vector.tensor_copy(qt[e][:, 0:512], qp0[:])
        nc.vector.tensor_copy(qt[e][:, 512:1024], qp1[:])

    # per-slot softmax chunk sums + norm scratch (cols 4j..4j+3 sums; 32+2j ssum; 33+2j rec)
    sums = sb_sums.tile([P, 48], dt.float32, tag="sums", name="sums")
    opart_d = sb_opart.tile([4 * P, D], dt.float32, tag="opartd", name="opartd")
    chunk_ctr = [0] * SLOTS

    for p in range(2):
        # ---- x^T for this pass's keys (direct DMA from host-transposed x)
        xt = [sb_xt.tile([P, HT * P], dt.float32r, tag=f"xt{d}", name=f"xt{d}")
              for d in range(ND)]
        for d in range(ND):
            for h in range(2):
                nc.sync.dma_start(xt[d][:, h * 512:(h + 1) * 512],
                                  xT[d * P:(d + 1) * P, p * 1024 + h * 512:p * 1024 + (h + 1) * 512])

        # ---- K^T projection for this pass
        kt = [sb_kt.tile([P, HT * P], dt.float32r, tag=f"kt{e}", name=f"kt{e}")
              for e in range(NE)]
        for e in range(NE):
            wb = sb_wblk.tile([P, D], dt.float32r, tag="wblk", name="wb")
            nc.sync.dma_start(wb[:], Wk[e * P:(e + 1) * P, :])
            _tg = "proj" if (p == 1 or e % 2 == 0) else "s"
            kp0 = ps.tile([P, 512], dt.float32, tag=_tg, name="kp0")
            kp1 = ps.tile([P, 512], dt.float32, tag=_tg, name="kp1")
            for d in range(ND):
                nc.tensor.matmul(kp0[:], wb[:, d * P:(d + 1) * P], xt[d][:, 0:512],
                                 start=(d == 0), stop=(d == ND - 1))
                nc.tensor.matmul(kp1[:], wb[:, d * P:(d + 1) * P], xt[d][:, 512:1024],
                                 start=(d == 0), stop=(d == ND - 1))
            nc.vector.tensor_copy(kt[e][:, 0:512], kp0[:])
            nc.vector.tensor_copy(kt[e][:, 512:1024], kp1[:])

        # ---- V projection for this pass (natural [s, e] layout)
        vt = [sb_vt.tile([P, D], dt.float32r, tag=f"vt{st}", name=f"vt{st}")
              for st in range(HT)]
        for ec in range(2):
            wv = sb_wv.tile([P, ND * 512], dt.float32r, tag="wv", name="wv")
            for d in range(ND):
                nc.sync.dma_start(wv[:, d * 512:(d + 1) * 512],
                                  Wv[ec * P:(ec + 1) * P, d * 512:(d + 1) * 512])
            for st in range(HT):
                vp = ps.tile([P, 512], dt.float32,
                             tag="proj" if (p == 1 or st % 2 == 0) else "o", name="vp")
                for d in range(ND):
                    nc.tensor.matmul(vp[:], xt[d][:, st * P:(st + 1) * P],
                                     wv[:, d * 512:(d + 1) * 512],
                                     start=(d == 0), stop=(d == ND - 1))
                nc.vector.tensor_copy(vt[st][:, ec * 512:(ec + 1) * 512], vp[:])

        # ---- attention for each slot against this pass's keys
        slot_order = list(range(SLOTS)) if p == 0 else [7, 6, 5, 4]
        for j in slot_order:
            ext = 2 * j + 2                      # causal extent in k-tiles (uniform)
            t0 = HT * p
            t1 = min(ext, HT * (p + 1))
            if t1 <= t0:
                continue
            final_pass = (ext - 1) // HT == p

            o_ps = [ps.tile([P, 512], dt.float32, tag="o", name=f"ops{ec}")
                    for ec in range(2)]

            c = t0 * P
            while c < t1 * P:
                cw = 512 if t1 * P - c >= 512 else t1 * P - c
                sps = ps.tile([P, cw], dt.float32, tag="s", name="sps")
                for e in range(NE):
                    nc.tensor.matmul(sps[:], qt[e][:, j * P:(j + 1) * P],
                                     kt[e][:, c - p * 1024: c - p * 1024 + cw],
                                     start=(e == 0), stop=(e == NE - 1))
                if final_pass and c + cw == ext * P:
                    off = cw - 2 * P
                    nc.vector.tensor_add(sps[:, off:off + 2 * P], sps[:, off:off + 2 * P], mask_sb[:])
                pch = sb_p.tile([P, cw], dt.float32r, tag="p", name="pch")
                ci = chunk_ctr[j]
                chunk_ctr[j] += 1
                nc.scalar.activation(pch[:], sps[:], AF.Exp, scale=SCALE,
                                     accum_out=sums[:, 4 * j + ci:4 * j + ci + 1])
                for ti in range(cw // P):
                    t = c // P + ti
                    ptp = ps.tile([P, P], dt.float32, tag="tr", name="ptp")
                    nc.tensor.transpose(ptp[:].bitcast(dt.float32r), pch[:, ti * P:(ti + 1) * P], ident_r[:])
                    ptb = sb_pt.tile([P, P], dt.float32r, tag="pt", name="ptb")
                    nc.vector.tensor_copy(ptb[:], ptp[:].bitcast(dt.float32r))
                    for ec in range(2):
                        nc.tensor.matmul(o_ps[ec][:], ptb[:],
                                         vt[t - HT * p][:, ec * 512:(ec + 1) * 512],
                                         start=(t == t0), stop=(t == t1 - 1))
                c += cw

            if final_pass:
                ssum = sums[:, 32 + 2 * j:33 + 2 * j]
                nc.vector.reduce_sum(ssum, sums[:, 4 * j:4 * j + chunk_ctr[j]],
                                     axis=mybir.AxisListType.X)
                rec = sums[:, 33 + 2 * j:34 + 2 * j]
                nc.vector.reciprocal(rec, ssum)
                ob = sb_obuf.tile([P, D], dt.float32, tag="obuf", name="ob")
                for ec in range(2):
                    if ext > HT:
                        oj = sb_ojoin.tile([P, 512], dt.float32, tag="ojoin", name="oj")
                        nc.sync.dma_start(oj[:], opart_d[(j - 4) * P:(j - 3) * P, ec * 512:(ec + 1) * 512])
                        nc.vector.tensor_add(oj[:], o_ps[ec][:], oj[:])
                        nc.scalar.mul(ob[:, ec * 512:(ec + 1) * 512], oj[:], rec)
                    else:
                        nc.scalar.mul(ob[:, ec * 512:(ec + 1) * 512], o_ps[ec][:], rec)
                nc.sync.dma_start(O[j * P:(j + 1) * P, :], ob[:])
            else:
                ob = sb_obuf.tile([P, D], dt.float32, tag="obuf", name="ob")
                for ec in range(2):
                    nc.vector.tensor_copy(ob[:, ec * 512:(ec + 1) * 512], o_ps[ec][:])
                nc.sync.dma_start(opart_d[(j - 4) * P:(j - 3) * P, :], ob[:])


def build_program(reps: int = 1):
    nc = bacc.Bacc("TRN2", target_bir_lowering=False, debug=False, num_devices=NCORES)

    xT_t = nc.dram_tensor("xT", [D, S], dt.float32r, kind="ExternalInput")
    xqT_t = nc.dram_tensor("xqT", [D, SLOTS * P], dt.float32r, kind="ExternalInput")
    Wq_t = nc.dram_tensor("Wq", [D, D], dt.float32r, kind="ExternalInput")
    Wk_t = nc.dram_tensor("Wk", [D, D], dt.float32r, kind="ExternalInput")
    Wv_t = nc.dram_tensor("Wv", [2 * P, ND * 512], dt.float32r, kind="ExternalInput")
    mask_t = nc.dram_tensor("mask", [P, 2 * P], dt.float32, kind="ExternalInput")
    ident_t = nc.dram_tensor("ident", [P, P], dt.float32r, kind="ExternalInput")
    O_t = nc.dram_tensor("O", [SLOTS * P, D], dt.float32, kind="ExternalOutput")

    aps = (xT_t.ap(), xqT_t.ap(), Wq_t.ap(), Wk_t.ap(), Wv_t.ap(), mask_t.ap(),
           ident_t.ap(), O_t.ap())

    with tile.TileContext(nc) as tc:
        with (
            tc.tile_pool(name="const", bufs=1) as sb_const,
            tc.tile_pool(name="xt", bufs=1) as sb_xt,
            tc.tile_pool(name="qt", bufs=1) as sb_qt,
            tc.tile_pool(name="kt", bufs=1) as sb_kt,
            tc.tile_pool(name="vt", bufs=1) as sb_vt,
            tc.tile_pool(name="wblk", bufs=3) as sb_wblk,
            tc.tile_pool(name="wv", bufs=1) as sb_wv,
            tc.tile_pool(name="p", bufs=3) as sb_p,
            tc.tile_pool(name="pt", bufs=6) as sb_pt,
            tc.tile_pool(name="sums", bufs=1) as sb_sums,
            tc.tile_pool(name="opart", bufs=1, space=bass.MemorySpace.DRAM) as sb_opart,
            tc.tile_pool(name="obuf", bufs=3) as sb_obuf,
            tc.tile_pool(name="ojoin", bufs=4) as sb_ojoin,
            tc.tile_pool(name="ps", bufs=2, space=bass.MemorySpace.PSUM) as ps,
        ):
            pools = (sb_const, sb_xt, sb_qt, sb_kt, sb_vt,
                     sb_wblk, sb_wv, sb_p, sb_pt, sb_sums, sb_opart,
                     sb_obuf, sb_ojoin, ps)
            if reps == 1:
                _emit_body(nc, tc, pools, aps)
            else:
                with tc.For_i(0, reps, 1):
                    _emit_body(nc, tc, pools, aps)

    nc.compile()
    return nc


def round_f32r(a):
    """Round fp32 to the fp32r grid (low 12 mantissa bits dropped, nearest-even)."""
    u = np.ascontiguousarray(a, np.float32).view(np.uint32).copy()
    low = u & np.uint32(0xFFF)
    base = u & np.uint32(0xFFFFF000)
    up = (low > 0x800) | ((low == 0x800) & (((base >> 12) & 1) == 1))
    base[up] += np.uint32(0x1000)
    return base.view(np.float32)


def pack_w_cols(W, cw):
    """[D, D] -> [(D//cw)*P, ND*cw]: row (e*P+p), col (d*cw+c) = W[d*P+p, e*cw+c]."""
    ne = D // cw
    return np.ascontiguousarray(
        W.reshape(ND, P, ne, cw).transpose(2, 1, 0, 3).reshape(ne * P, ND * cw))


def make_in_maps(x, Wq, Wk, Wv):
    x = round_f32r(x.reshape(B, S, D))
    Wq = pack_w_cols(round_f32r(Wq), P)
    Wk = pack_w_cols(round_f32r(Wk), P)
    Wv = pack_w_cols(round_f32r(Wv), 512)
    ident = np.eye(P, dtype=np.float32)
    tri = np.where(np.arange(P)[None, :] <= np.arange(P)[:, None], 0.0, NEG).astype(np.float32)
    masks = [
        np.concatenate([tri, np.full((P, P), NEG, np.float32)], axis=1),   # parity 0
        np.concatenate([np.zeros((P, P), np.float32), tri], axis=1),       # parity 1
    ]
    xT = [np.ascontiguousarray(x[b].T) for b in range(B)]   # [D, S] per batch
    in_maps = []
    for c in range(NCORES):
        b, r = c // 2, c % 2
        xTb = xT[b]
        cols = np.concatenate([np.arange((2 * j + r) * P, (2 * j + r + 1) * P)
                               for j in range(SLOTS)])
        xqTb = np.ascontiguousarray(xTb[:, cols])
        in_maps.append({
            "xT": xTb, "xqT": xqTb, "Wq": Wq, "Wk": Wk, "Wv": Wv,
            "mask": masks[r], "ident": ident,
        })
    return in_maps


def assemble_output(results):
    out = np.empty((B, S, D), dtype=np.float32)
    for c in range(NCORES):
        b, r = c // 2, c % 2
        oc = results[c]["O"].reshape(SLOTS, P, D)
        for j in range(SLOTS):
            out[b, (2 * j + r) * P:(2 * j + r + 1) * P, :] = oc[j]
    return out


_nc_cache = {}


def _get_program(reps: int = 1):
    if reps not in _nc_cache:
        _nc_cache[reps] = build_program(reps)
    return _nc_cache[reps]


def kernel(x, Wq, Wk, Wv):
    x = np.asarray(x, dtype=np.float32)
    Wq = np.asarray(Wq, dtype=np.float32)
    Wk = np.asarray(Wk, dtype=np.float32)
    Wv = np.asarray(Wv, dtype=np.float32)
    nc = _get_program(1)
    in_maps = make_in_maps(x, Wq, Wk, Wv)
    results = run_bass_kernel_spmd(nc, in_maps, list(range(NCORES))).results
    return assemble_output(results)
